# revision 18
# baseline (speedup 1.0000x reference)
"""Trainium2 Bass kernel for sliding-window GQA attention block.

Reference computation (B=2, S=4096, DIM=1024, H=16 q-heads, KV=2 kv-heads,
D=64, W=256 window):
    q = x@Wq + bq ; k = x@Wk + bk ; v = x@Wv + bv        (GQA repeat kv x8)
    local attention: query t attends keys [t-128, t+128) (zero-padded edges,
    no 1/sqrt(d) scaling), softmax, out = probs@v
    y = out@Wo + bo

Sharding: 8 cores = batch(2) x seq-quarter(4). Each core computes 1024
query rows end-to-end (all 16 heads) from a 1280-row haloed x slice.
No cross-core communication; host pads/transposes/gathers.

On-device pipeline per core (all matmuls bf16, fp32 PSUM accumulation):
  QKV projections -> scores (queries on partitions, row-packed head pairs,
  additive -1e30 band masks via identity-matmuls) -> exp with fused
  per-partition accumulate (softmax denominators) -> PE-transpose of probs
  -> probs@V -> divide by denominator (per-partition scalar) -> transpose
  -> out-projection with K=1 bias-row fold.
"""

import functools
import numpy as np

B, S, DIM = 2, 4096, 1024
H, KV, D = 16, 2, 64
W, HW = 256, 128
NCORES = 8
QT = 4           # sequence quarters
T = S // QT      # 1024 query rows per core
TH = T + 2 * HW  # 1280 haloed rows
NEG = -1e30


@functools.lru_cache(maxsize=1)
def _build_nc():
    import concourse.bacc as bacc
    import concourse.tile as tile
    from concourse import mybir
    from concourse.masks import make_identity

    f32 = mybir.dt.float32
    bf16 = mybir.dt.bfloat16
    Exp = mybir.ActivationFunctionType.Exp
    Identity = mybir.ActivationFunctionType.Identity

    nc = bacc.Bacc("TRN2", target_bir_lowering=False, debug=False)

    xT = nc.dram_tensor("xT", [DIM, TH], bf16, kind="ExternalInput")
    wq = nc.dram_tensor("Wq", [DIM, DIM], bf16, kind="ExternalInput")
    wk = nc.dram_tensor("Wk", [DIM, KV * D], bf16, kind="ExternalInput")
    wv = nc.dram_tensor("Wv", [DIM, KV * D], bf16, kind="ExternalInput")
    wo = nc.dram_tensor("Wo", [DIM, DIM], bf16, kind="ExternalInput")
    bqc = nc.dram_tensor("bqc", [128, 8], f32, kind="ExternalInput")
    bk_row = nc.dram_tensor("bk_row", [1, KV * D], bf16, kind="ExternalInput")
    bv_row = nc.dram_tensor("bv_row", [1, KV * D], bf16, kind="ExternalInput")
    bo_row = nc.dram_tensor("bo_row", [1, DIM], bf16, kind="ExternalInput")
    ind = nc.dram_tensor("ind", [1, TH], bf16, kind="ExternalInput")
    out = nc.dram_tensor("out", [T, DIM], f32, kind="ExternalOutput")

    with tile.TileContext(nc) as tc:
        with tc.tile_pool(name="const", bufs=1) as const, \
             tc.tile_pool(name="w", bufs=1) as wpool, \
             tc.tile_pool(name="act", bufs=1) as actp, \
             tc.tile_pool(name="attn", bufs=2) as attnp, \
             tc.tile_pool(name="ps", bufs=2, space="PSUM") as ps:

            # ---- constants -------------------------------------------------
            ident = const.tile([128, 128], bf16, tag="ident")
            make_identity(nc, ident)
            # 0/1 window mask in transposed (key-partition r, query-col c)
            # orientation, chunks j=0..2 of the 384-wide score block:
            # chunk0 valid where r >= c, chunk1 all-valid, chunk2 valid
            # where r < c. Multiplied into exp(scores^T) on the DVE.
            mask01T = const.tile([128, 384], bf16, tag="mask01T")
            nc.gpsimd.memset(mask01T, 1.0)
            nc.gpsimd.affine_select(
                out=mask01T[:, 0:128], in_=mask01T[:, 0:128],
                compare_op=mybir.AluOpType.is_ge,
                fill=0.0, base=0, pattern=[[-1, 128]], channel_multiplier=1)
            nc.gpsimd.affine_select(
                out=mask01T[:, 256:384], in_=mask01T[:, 256:384],
                compare_op=mybir.AluOpType.is_ge,
                fill=0.0, base=-1, pattern=[[1, 128]], channel_multiplier=-1)
            ones_row = const.tile([1, 128], bf16, tag="ones")
            nc.vector.memset(ones_row, 1.0)

            bq_sb = const.tile([128, 8], f32, tag="bq")
            nc.sync.dma_start(out=bq_sb, in_=bqc[:, :])
            bkr = const.tile([1, KV * D], bf16, tag="bkr")
            nc.sync.dma_start(out=bkr, in_=bk_row[:, :])
            bvr = const.tile([1, KV * D], bf16, tag="bvr")
            nc.sync.dma_start(out=bvr, in_=bv_row[:, :])
            bor = const.tile([1, DIM], bf16, tag="bor")
            nc.sync.dma_start(out=bor, in_=bo_row[:, :])
            ind_sb = const.tile([1, TH], bf16, tag="ind")
            nc.sync.dma_start(out=ind_sb, in_=ind[:, :])

            # ---- weight/activation loads ----------------------------------
            xT_sb = []
            wq_sb, wk_sb, wv_sb, wo_sb = [], [], [], []
            for k in range(8):
                eng = nc.sync if k % 2 == 0 else nc.scalar
                t_x = wpool.tile([128, TH], bf16, tag=f"xT{k}", name=f"xT{k}")
                eng.dma_start(out=t_x, in_=xT[k * 128:(k + 1) * 128, :])
                xT_sb.append(t_x)
                t_k = wpool.tile([128, KV * D], bf16, tag=f"wk{k}", name=f"wk{k}")
                eng.dma_start(out=t_k, in_=wk[k * 128:(k + 1) * 128, :])
                wk_sb.append(t_k)
                t_v = wpool.tile([128, KV * D], bf16, tag=f"wv{k}", name=f"wv{k}")
                eng.dma_start(out=t_v, in_=wv[k * 128:(k + 1) * 128, :])
                wv_sb.append(t_v)
            for k in range(8):
                t_q = wpool.tile([128, DIM], bf16, tag=f"wq{k}", name=f"wq{k}")
                (nc.sync if k % 2 == 0 else nc.scalar).dma_start(
                    out=t_q, in_=wq[k * 128:(k + 1) * 128, :])
                wq_sb.append(t_q)
            for k in range(8):
                t_o = wpool.tile([128, DIM], bf16, tag=f"wo{k}", name=f"wo{k}")
                (nc.sync if k % 2 == 0 else nc.scalar).dma_start(
                    out=t_o, in_=wo[k * 128:(k + 1) * 128, :])
                wo_sb.append(t_o)

            # ---- Q projection: qT[m] holds heads (m, m+8) on partition
            # halves (row-packed pairs for the scores matmuls) --------------
            qT_sb = []
            for m in range(8):
                t_qt = actp.tile([128, T], bf16, tag=f"qT{m}", name=f"qT{m}")
                qT_sb.append(t_qt)
            def q_proj(m, n):
                q_ps = ps.tile([128, 512], f32, tag="proj", name="q_ps")
                for k in range(8):
                    nc.tensor.matmul(
                        out=q_ps,
                        lhsT=wq_sb[k][:, m * 128:(m + 1) * 128],
                        rhs=xT_sb[k][:, HW + n * 512: HW + (n + 1) * 512],
                        start=(k == 0), stop=(k == 7))
                nc.scalar.activation(
                    out=qT_sb[m][:, n * 512:(n + 1) * 512], in_=q_ps,
                    func=Identity, bias=bq_sb[:, m:m + 1], scale=1.0)

            # ---- K projection over halo; zero at padded rows via ind fold -
            kT_sb = actp.tile([128, TH], bf16, tag="kT")

            def k_proj(c0, cw):
                k_ps = ps.tile([128, 512], f32, tag="proj", name="k_ps")
                for k in range(8):
                    nc.tensor.matmul(
                        out=k_ps[:, :cw], lhsT=wk_sb[k],
                        rhs=xT_sb[k][:, c0:c0 + cw],
                        start=(k == 0), stop=False)
                nc.tensor.matmul(
                    out=k_ps[:, :cw], lhsT=bkr, rhs=ind_sb[:, c0:c0 + cw],
                    start=False, stop=True)
                nc.scalar.copy(out=kT_sb[:, c0:c0 + cw], in_=k_ps[:, :cw])

            k_proj(0, 512)
            k_proj(512, 512)

            # ---- V projection (keys on partitions). Layout per u-tile is
            # [V_kv0 (64) | 1 | V_kv1 (64) | 1]: the ones column appended to
            # each kv-slice makes the probs@[V|1] matmul emit the softmax
            # denominator as output column 64 for free. ---------------------
            NU = TH // 128
            v_sb = actp.tile([128, NU * 130], bf16, tag="V")
            v_view = v_sb.rearrange("p (u g c) -> p u g c", u=NU, g=2)
            nc.vector.memset(v_view[:, :, :, 64:65], 1.0)
            def v_proj(ut):
                v_ps = ps.tile([128, 512], f32, tag="proj", name="v_ps")
                for k in range(8):
                    nc.tensor.matmul(
                        out=v_ps[:, :KV * D],
                        lhsT=xT_sb[k][:, ut * 128:(ut + 1) * 128],
                        rhs=wv_sb[k], start=(k == 0), stop=False)
                nc.tensor.matmul(
                    out=v_ps[:, :KV * D],
                    lhsT=ind_sb[:, ut * 128:(ut + 1) * 128], rhs=bvr,
                    start=False, stop=True)
                nc.vector.tensor_copy(
                    out=v_view[:, ut, :, 0:64],
                    in_=v_ps[:, :KV * D].rearrange("p (g c) -> p g c", g=2))

            for ut in range(5):
                v_proj(ut)

            for m in range(8):
                q_proj(m, 0)

            # ---- attention + output transpose -----------------------------
            attnT = actp.tile([128, 8 * T], bf16, tag="attnT")
            attnT_v = attnT.rearrange("p (k t) -> p k t", k=8)
            for blk in range(4):
                for tt in range(2):
                    qcol = blk * 256 + tt * 128
                    u0 = qcol  # halo col of first attended key
                    attn_t = attnp.tile([128, DIM], bf16, tag="attn", bufs=3)
                    for mg in range(0, 8, 2):
                        # 4 heads per group: (mg, mg+1) x (kv0, kv1); scores
                        # ordered so consecutive matmuls share their
                        # stationary kT chunk, and the two kv halves run on
                        # distinct PE row-groups concurrently.
                        s_pss = {}
                        for m in (mg, mg + 1):
                            for half in range(2):
                                s_pss[(m, half)] = ps.tile(
                                    [128, 384], f32, tag="s", bufs=4,
                                    name="s_ps")
                        for j in range(3):
                            for half in range(2):
                                for m in (mg, mg + 1):
                                    nc.tensor.matmul(
                                        out=s_pss[(m, half)][:, j * 128:
                                                             (j + 1) * 128],
                                        lhsT=kT_sb[half * 64:(half + 1) * 64,
                                                   u0 + j * 128:
                                                   u0 + (j + 1) * 128],
                                        rhs=qT_sb[m][half * 64:(half + 1) * 64,
                                                     qcol:qcol + 128],
                                        start=(j == 0), stop=(j == 2),
                                        tile_position=(64 * half, 0))
                        for m in (mg, mg + 1):
                            for half in range(2):
                                h = m + 8 * half
                                p_raw = attnp.tile([128, 384], bf16,
                                                   tag="Praw", bufs=6,
                                                   name="p_raw")
                                nc.scalar.activation(out=p_raw,
                                                     in_=s_pss[(m, half)],
                                                     func=Exp)
                                p_t = attnp.tile([128, 384], bf16, tag="P",
                                                 bufs=6, name="p_t")
                                nc.vector.tensor_mul(p_t, p_raw, mask01T)
                                o_ps = ps.tile([128, 65], f32, tag="o",
                                               name="o_ps")
                                for j in range(3):
                                    ut = blk * 2 + tt + j
                                    nc.tensor.matmul(
                                        out=o_ps,
                                        lhsT=p_t[:, j * 128:(j + 1) * 128],
                                        rhs=v_view[:, ut, half, 0:65],
                                        start=(j == 0), stop=(j == 2))
                                rc = attnp.tile([128, 1], f32, tag="rc",
                                                bufs=4, name="rc")
                                nc.vector.reciprocal(out=rc,
                                                     in_=o_ps[:, 64:65])
                                nc.vector.tensor_scalar_mul(
                                    attn_t[:, h * 64:(h + 1) * 64],
                                    o_ps[:, 0:64], rc)
                    # transpose attn rows (t) x cols (hd) -> attnT k-tiles
                    for g in range(3):
                        kcnt = 3 if g < 2 else 2
                        at_ps = ps.tile([128, 384], bf16, tag="o",
                                        name="at_ps")
                        for jj in range(kcnt):
                            kk = g * 3 + jj
                            nc.tensor.matmul(
                                out=at_ps[:, jj * 128:(jj + 1) * 128],
                                lhsT=attn_t[:, kk * 128:(kk + 1) * 128],
                                rhs=ident, is_transpose=True,
                                start=(jj == 0), stop=(jj == kcnt - 1))
                        src = at_ps[:, :kcnt * 128].rearrange(
                            "p (j c) -> p j c", j=kcnt)
                        dst = attnT_v[:, g * 3:g * 3 + kcnt, qcol:qcol + 128]
                        if tt == 0:
                            nc.scalar.copy(out=dst, in_=src)
                        else:
                            nc.vector.tensor_copy(out=dst, in_=src)

                    # ---- output projection for this query tile (keeps the
                    # PE fed with dense matmuls between attention phases) ----
                    mt = blk * 2 + tt
                    out_t = attnp.tile([128, DIM], f32, tag="outt")
                    o2s = [ps.tile([128, 512], f32, tag="proj", name="o2_ps")
                           for _ in range(2)]
                    for k in range(8):
                        for n in range(2):
                            nc.tensor.matmul(
                                out=o2s[n],
                                lhsT=attnT[:, k * T + mt * 128:
                                           k * T + (mt + 1) * 128],
                                rhs=wo_sb[k][:, n * 512:(n + 1) * 512],
                                start=(k == 0), stop=False)
                    for n in range(2):
                        nc.tensor.matmul(
                            out=o2s[n], lhsT=ones_row,
                            rhs=bor[:, n * 512:(n + 1) * 512],
                            start=False, stop=True)
                        nc.scalar.copy(out=out_t[:, n * 512:(n + 1) * 512],
                                       in_=o2s[n])
                    nc.sync.dma_start(out=out[mt * 128:(mt + 1) * 128, :],
                                      in_=out_t)
                    # just-in-time projection work: keeps dense matmuls
                    # flowing between attention tiles (HAM stays warm)
                    if mt == 0:
                        v_proj(5)
                        q_proj(0, 1)
                        q_proj(1, 1)
                        q_proj(2, 1)
                        q_proj(3, 1)
                    elif mt == 1:
                        v_proj(6)
                        q_proj(4, 1)
                        q_proj(5, 1)
                        q_proj(6, 1)
                        q_proj(7, 1)
                    elif mt == 2:
                        v_proj(7)
                        k_proj(1024, 256)
                    elif mt == 3:
                        v_proj(8)
                    elif mt == 4:
                        v_proj(9)

    nc.compile()
    return nc


def _host_prep(x, Wq, bq, Wk, bk, Wv, bv, Wo, bo):
    import ml_dtypes
    bf16 = ml_dtypes.bfloat16

    # permute Wq/bq columns so qT m-tile holds head m on partitions 0-63 and
    # head m+8 on partitions 64-127 (enables row-packed score matmuls)
    idx = np.empty(DIM, dtype=np.int64)
    for m in range(8):
        for j in range(128):
            h = m if j < 64 else m + 8
            idx[m * 128 + j] = h * D + (j % 64)
    wq_p = np.ascontiguousarray(Wq[:, idx]).astype(bf16)
    bq_p = bq[idx].astype(np.float32).reshape(8, 128).T.copy()  # (128, 8)
    wk_b = np.ascontiguousarray(Wk).astype(bf16)
    wv_b = np.ascontiguousarray(Wv).astype(bf16)
    wo_b = np.ascontiguousarray(Wo).astype(bf16)
    bk_r = bk.reshape(1, KV * D).astype(bf16)
    bv_r = bv.reshape(1, KV * D).astype(bf16)
    bo_r = bo.reshape(1, DIM).astype(bf16)

    in_maps = []
    for c in range(NCORES):
        b, qt = c // QT, c % QT
        lo, hi = qt * T - HW, qt * T + T + HW
        xs = np.zeros((TH, DIM), dtype=np.float32)
        s0, s1 = max(lo, 0), min(hi, S)
        xs[s0 - lo:s1 - lo] = x[b, s0:s1]
        ind_r = np.zeros((1, TH), dtype=bf16)
        ind_r[0, s0 - lo:s1 - lo] = 1.0
        in_maps.append({
            "xT": np.ascontiguousarray(xs.T).astype(bf16),
            "Wq": wq_p, "Wk": wk_b, "Wv": wv_b, "Wo": wo_b,
            "bqc": bq_p, "bk_row": bk_r, "bv_row": bv_r, "bo_row": bo_r,
            "ind": ind_r,
        })
    return in_maps


def kernel(x, Wq, bq, Wk, bk, Wv, bv, Wo, bo):
    from concourse.bass_utils import run_bass_kernel_spmd

    nc = _build_nc()
    in_maps = _host_prep(x, Wq, bq, Wk, bk, Wv, bv, Wo, bo)
    res = run_bass_kernel_spmd(nc, in_maps, core_ids=list(range(NCORES)))
    out = np.empty((B, S, DIM), dtype=np.float32)
    for c in range(NCORES):
        b, qt = c // QT, c % QT
        out[b, qt * T:(qt + 1) * T] = res.results[c]["out"]
    return out


# revision 19
# speedup vs baseline: 1.0229x; 1.0229x over previous
"""Trainium2 Bass kernel for sliding-window GQA attention block.

Reference computation (B=2, S=4096, DIM=1024, H=16 q-heads, KV=2 kv-heads,
D=64, W=256 window):
    q = x@Wq + bq ; k = x@Wk + bk ; v = x@Wv + bv        (GQA repeat kv x8)
    local attention: query t attends keys [t-128, t+128) (zero-padded edges,
    no 1/sqrt(d) scaling), softmax, out = probs@v
    y = out@Wo + bo

Sharding: 8 cores = batch(2) x seq-quarter(4). Each core computes 1024
query rows end-to-end (all 16 heads) from a 1280-row haloed x slice.
No cross-core communication; host pads/transposes/gathers.

On-device pipeline per core (all matmuls bf16, fp32 PSUM accumulation):
  QKV projections -> scores (queries on partitions, row-packed head pairs,
  additive -1e30 band masks via identity-matmuls) -> exp with fused
  per-partition accumulate (softmax denominators) -> PE-transpose of probs
  -> probs@V -> divide by denominator (per-partition scalar) -> transpose
  -> out-projection with K=1 bias-row fold.
"""

import functools
import numpy as np

B, S, DIM = 2, 4096, 1024
H, KV, D = 16, 2, 64
W, HW = 256, 128
NCORES = 8
QT = 4           # sequence quarters
T = S // QT      # 1024 query rows per core
TH = T + 2 * HW  # 1280 haloed rows
NEG = -1e30


@functools.lru_cache(maxsize=1)
def _build_nc():
    import concourse.bacc as bacc
    import concourse.tile as tile
    from concourse import mybir
    from concourse.masks import make_identity

    f32 = mybir.dt.float32
    bf16 = mybir.dt.bfloat16
    Exp = mybir.ActivationFunctionType.Exp
    Identity = mybir.ActivationFunctionType.Identity

    nc = bacc.Bacc("TRN2", target_bir_lowering=False, debug=False)

    xT = nc.dram_tensor("xT", [DIM, TH], bf16, kind="ExternalInput")
    wq = nc.dram_tensor("Wq", [DIM, DIM], bf16, kind="ExternalInput")
    wk = nc.dram_tensor("Wk", [DIM, KV * D], bf16, kind="ExternalInput")
    wv = nc.dram_tensor("Wv", [DIM, KV * D], bf16, kind="ExternalInput")
    wo = nc.dram_tensor("Wo", [DIM, DIM], bf16, kind="ExternalInput")
    bqc = nc.dram_tensor("bqc", [128, 8], f32, kind="ExternalInput")
    bk_row = nc.dram_tensor("bk_row", [1, KV * D], bf16, kind="ExternalInput")
    bv_row = nc.dram_tensor("bv_row", [1, KV * D], bf16, kind="ExternalInput")
    bo_row = nc.dram_tensor("bo_row", [1, DIM], bf16, kind="ExternalInput")
    ind = nc.dram_tensor("ind", [1, TH], bf16, kind="ExternalInput")
    out = nc.dram_tensor("out", [T, DIM], f32, kind="ExternalOutput")

    with tile.TileContext(nc) as tc:
        with tc.tile_pool(name="const", bufs=1) as const, \
             tc.tile_pool(name="w", bufs=1) as wpool, \
             tc.tile_pool(name="act", bufs=1) as actp, \
             tc.tile_pool(name="attn", bufs=2) as attnp, \
             tc.tile_pool(name="ps", bufs=2, space="PSUM") as ps:

            # ---- constants -------------------------------------------------
            ident = const.tile([128, 128], bf16, tag="ident")
            make_identity(nc, ident)
            # 0/1 window mask in transposed (key-partition r, query-col c)
            # orientation, chunks j=0..2 of the 384-wide score block:
            # chunk0 valid where r >= c, chunk1 all-valid, chunk2 valid
            # where r < c. Multiplied into exp(scores^T) on the DVE.
            mask01T = const.tile([128, 384], bf16, tag="mask01T")
            nc.gpsimd.memset(mask01T, 1.0)
            nc.gpsimd.affine_select(
                out=mask01T[:, 0:128], in_=mask01T[:, 0:128],
                compare_op=mybir.AluOpType.is_ge,
                fill=0.0, base=0, pattern=[[-1, 128]], channel_multiplier=1)
            nc.gpsimd.affine_select(
                out=mask01T[:, 256:384], in_=mask01T[:, 256:384],
                compare_op=mybir.AluOpType.is_ge,
                fill=0.0, base=-1, pattern=[[1, 128]], channel_multiplier=-1)
            ones_row = const.tile([1, 128], bf16, tag="ones")
            nc.vector.memset(ones_row, 1.0)

            bq_sb = const.tile([128, 8], f32, tag="bq")
            nc.sync.dma_start(out=bq_sb, in_=bqc[:, :])
            bkr = const.tile([1, KV * D], bf16, tag="bkr")
            nc.sync.dma_start(out=bkr, in_=bk_row[:, :])
            bvr = const.tile([1, KV * D], bf16, tag="bvr")
            nc.sync.dma_start(out=bvr, in_=bv_row[:, :])
            bor = const.tile([1, DIM], bf16, tag="bor")
            nc.sync.dma_start(out=bor, in_=bo_row[:, :])
            ind_sb = const.tile([1, TH], bf16, tag="ind")
            nc.sync.dma_start(out=ind_sb, in_=ind[:, :])

            # ---- weight/activation loads ----------------------------------
            xT_sb = []
            wq_sb, wk_sb, wv_sb, wo_sb = [], [], [], []
            for k in range(8):
                eng = nc.sync if k % 2 == 0 else nc.scalar
                t_x = wpool.tile([128, TH], bf16, tag=f"xT{k}", name=f"xT{k}")
                eng.dma_start(out=t_x, in_=xT[k * 128:(k + 1) * 128, :])
                xT_sb.append(t_x)
                t_k = wpool.tile([128, KV * D], bf16, tag=f"wk{k}", name=f"wk{k}")
                eng.dma_start(out=t_k, in_=wk[k * 128:(k + 1) * 128, :])
                wk_sb.append(t_k)
                t_v = wpool.tile([128, KV * D], bf16, tag=f"wv{k}", name=f"wv{k}")
                eng.dma_start(out=t_v, in_=wv[k * 128:(k + 1) * 128, :])
                wv_sb.append(t_v)
            for k in range(8):
                t_q = wpool.tile([128, DIM], bf16, tag=f"wq{k}", name=f"wq{k}")
                (nc.sync if k % 2 == 0 else nc.scalar).dma_start(
                    out=t_q, in_=wq[k * 128:(k + 1) * 128, :])
                wq_sb.append(t_q)
            for k in range(8):
                t_o = wpool.tile([128, DIM], bf16, tag=f"wo{k}", name=f"wo{k}")
                (nc.sync if k % 2 == 0 else nc.scalar).dma_start(
                    out=t_o, in_=wo[k * 128:(k + 1) * 128, :])
                wo_sb.append(t_o)

            # ---- Q projection: qT[m] holds heads (m, m+8) on partition
            # halves (row-packed pairs for the scores matmuls) --------------
            qT_sb = []
            for m in range(8):
                t_qt = actp.tile([128, T], bf16, tag=f"qT{m}", name=f"qT{m}")
                qT_sb.append(t_qt)
            def q_proj(m, n):
                q_ps = ps.tile([128, 512], f32, tag="proj", name="q_ps")
                for k in range(8):
                    nc.tensor.matmul(
                        out=q_ps,
                        lhsT=wq_sb[k][:, m * 128:(m + 1) * 128],
                        rhs=xT_sb[k][:, HW + n * 512: HW + (n + 1) * 512],
                        start=(k == 0), stop=(k == 7))
                nc.scalar.activation(
                    out=qT_sb[m][:, n * 512:(n + 1) * 512], in_=q_ps,
                    func=Identity, bias=bq_sb[:, m:m + 1], scale=1.0)

            # ---- K projection over halo; zero at padded rows via ind fold -
            kT_sb = actp.tile([128, TH], bf16, tag="kT")

            def k_proj(c0, cw):
                k_ps = ps.tile([128, 512], f32, tag="proj", name="k_ps")
                for k in range(8):
                    nc.tensor.matmul(
                        out=k_ps[:, :cw], lhsT=wk_sb[k],
                        rhs=xT_sb[k][:, c0:c0 + cw],
                        start=(k == 0), stop=False)
                nc.tensor.matmul(
                    out=k_ps[:, :cw], lhsT=bkr, rhs=ind_sb[:, c0:c0 + cw],
                    start=False, stop=True)
                nc.scalar.copy(out=kT_sb[:, c0:c0 + cw], in_=k_ps[:, :cw])

            k_proj(0, 512)
            k_proj(512, 512)

            # ---- V projection (keys on partitions). Layout per u-tile is
            # [V_kv0 (64) | 1 | V_kv1 (64) | 1]: the ones column appended to
            # each kv-slice makes the probs@[V|1] matmul emit the softmax
            # denominator as output column 64 for free. ---------------------
            NU = TH // 128
            v_sb = actp.tile([128, NU * 130], bf16, tag="V")
            v_view = v_sb.rearrange("p (u g c) -> p u g c", u=NU, g=2)
            nc.vector.memset(v_view[:, :, :, 64:65], 1.0)
            def v_proj(ut):
                v_ps = ps.tile([128, 512], f32, tag="proj", name="v_ps")
                for k in range(8):
                    nc.tensor.matmul(
                        out=v_ps[:, :KV * D],
                        lhsT=xT_sb[k][:, ut * 128:(ut + 1) * 128],
                        rhs=wv_sb[k], start=(k == 0), stop=False)
                nc.tensor.matmul(
                    out=v_ps[:, :KV * D],
                    lhsT=ind_sb[:, ut * 128:(ut + 1) * 128], rhs=bvr,
                    start=False, stop=True)
                nc.vector.tensor_copy(
                    out=v_view[:, ut, :, 0:64],
                    in_=v_ps[:, :KV * D].rearrange("p (g c) -> p g c", g=2))

            for ut in range(5):
                v_proj(ut)

            for m in range(8):
                q_proj(m, 0)

            # ---- attention + output transpose -----------------------------
            attnT = actp.tile([128, 8 * T], bf16, tag="attnT")
            attnT_v = attnT.rearrange("p (k t) -> p k t", k=8)
            for blk in range(4):
                for tt in range(2):
                    qcol = blk * 256 + tt * 128
                    u0 = qcol  # halo col of first attended key
                    attn_t = attnp.tile([128, DIM], bf16, tag="attn")
                    for mg in range(0, 8, 2):
                        # 4 heads per group: (mg, mg+1) x (kv0, kv1); scores
                        # ordered so consecutive matmuls share their
                        # stationary kT chunk, and the two kv halves run on
                        # distinct PE row-groups concurrently.
                        s_pss = {}
                        for m in (mg, mg + 1):
                            for half in range(2):
                                s_pss[(m, half)] = ps.tile(
                                    [128, 384], f32, tag="s", bufs=4,
                                    name="s_ps")
                        for j in range(3):
                            for half in range(2):
                                for m in (mg, mg + 1):
                                    nc.tensor.matmul(
                                        out=s_pss[(m, half)][:, j * 128:
                                                             (j + 1) * 128],
                                        lhsT=kT_sb[half * 64:(half + 1) * 64,
                                                   u0 + j * 128:
                                                   u0 + (j + 1) * 128],
                                        rhs=qT_sb[m][half * 64:(half + 1) * 64,
                                                     qcol:qcol + 128],
                                        start=(j == 0), stop=(j == 2),
                                        tile_position=(64 * half, 0))
                        for m in (mg, mg + 1):
                            for half in range(2):
                                h = m + 8 * half
                                p_raw = attnp.tile([128, 384], bf16,
                                                   tag="Praw", bufs=4,
                                                   name="p_raw")
                                nc.scalar.activation(out=p_raw,
                                                     in_=s_pss[(m, half)],
                                                     func=Exp)
                                p_t = attnp.tile([128, 384], bf16, tag="P",
                                                 bufs=4, name="p_t")
                                nc.vector.tensor_mul(p_t, p_raw, mask01T)
                                o_ps = ps.tile([128, 65], f32, tag="o",
                                               name="o_ps")
                                for j in range(3):
                                    ut = blk * 2 + tt + j
                                    nc.tensor.matmul(
                                        out=o_ps,
                                        lhsT=p_t[:, j * 128:(j + 1) * 128],
                                        rhs=v_view[:, ut, half, 0:65],
                                        start=(j == 0), stop=(j == 2))
                                rc = attnp.tile([128, 1], f32, tag="rc",
                                                bufs=4, name="rc")
                                nc.vector.reciprocal(out=rc,
                                                     in_=o_ps[:, 64:65])
                                nc.vector.tensor_scalar_mul(
                                    attn_t[:, h * 64:(h + 1) * 64],
                                    o_ps[:, 0:64], rc)
                    # transpose attn rows (t) x cols (hd) -> attnT k-tiles
                    for g in range(3):
                        kcnt = 3 if g < 2 else 2
                        at_ps = ps.tile([128, 384], bf16, tag="o",
                                        name="at_ps")
                        for jj in range(kcnt):
                            kk = g * 3 + jj
                            nc.tensor.matmul(
                                out=at_ps[:, jj * 128:(jj + 1) * 128],
                                lhsT=attn_t[:, kk * 128:(kk + 1) * 128],
                                rhs=ident, is_transpose=True,
                                start=(jj == 0), stop=(jj == kcnt - 1))
                        src = at_ps[:, :kcnt * 128].rearrange(
                            "p (j c) -> p j c", j=kcnt)
                        dst = attnT_v[:, g * 3:g * 3 + kcnt, qcol:qcol + 128]
                        if tt == 0:
                            nc.scalar.copy(out=dst, in_=src)
                        else:
                            nc.vector.tensor_copy(out=dst, in_=src)

                    # ---- output projection for this query tile (keeps the
                    # PE fed with dense matmuls between attention phases) ----
                    mt = blk * 2 + tt
                    out_t = attnp.tile([128, DIM], f32, tag="outt")
                    o2s = [ps.tile([128, 512], f32, tag="proj", name="o2_ps")
                           for _ in range(2)]
                    for k in range(8):
                        for n in range(2):
                            nc.tensor.matmul(
                                out=o2s[n],
                                lhsT=attnT[:, k * T + mt * 128:
                                           k * T + (mt + 1) * 128],
                                rhs=wo_sb[k][:, n * 512:(n + 1) * 512],
                                start=(k == 0), stop=False)
                    for n in range(2):
                        nc.tensor.matmul(
                            out=o2s[n], lhsT=ones_row,
                            rhs=bor[:, n * 512:(n + 1) * 512],
                            start=False, stop=True)
                        nc.scalar.copy(out=out_t[:, n * 512:(n + 1) * 512],
                                       in_=o2s[n])
                    nc.sync.dma_start(out=out[mt * 128:(mt + 1) * 128, :],
                                      in_=out_t)
                    # just-in-time projection work: keeps dense matmuls
                    # flowing between attention tiles (HAM stays warm)
                    if mt == 0:
                        v_proj(5)
                        q_proj(0, 1)
                        q_proj(1, 1)
                        q_proj(2, 1)
                        q_proj(3, 1)
                    elif mt == 1:
                        v_proj(6)
                        q_proj(4, 1)
                        q_proj(5, 1)
                        q_proj(6, 1)
                        q_proj(7, 1)
                    elif mt == 2:
                        v_proj(7)
                        k_proj(1024, 256)
                    elif mt == 3:
                        v_proj(8)
                    elif mt == 4:
                        v_proj(9)

    nc.compile()
    return nc


def _host_prep(x, Wq, bq, Wk, bk, Wv, bv, Wo, bo):
    import ml_dtypes
    bf16 = ml_dtypes.bfloat16

    # permute Wq/bq columns so qT m-tile holds head m on partitions 0-63 and
    # head m+8 on partitions 64-127 (enables row-packed score matmuls)
    idx = np.empty(DIM, dtype=np.int64)
    for m in range(8):
        for j in range(128):
            h = m if j < 64 else m + 8
            idx[m * 128 + j] = h * D + (j % 64)
    wq_p = np.ascontiguousarray(Wq[:, idx]).astype(bf16)
    bq_p = bq[idx].astype(np.float32).reshape(8, 128).T.copy()  # (128, 8)
    wk_b = np.ascontiguousarray(Wk).astype(bf16)
    wv_b = np.ascontiguousarray(Wv).astype(bf16)
    wo_b = np.ascontiguousarray(Wo).astype(bf16)
    bk_r = bk.reshape(1, KV * D).astype(bf16)
    bv_r = bv.reshape(1, KV * D).astype(bf16)
    bo_r = bo.reshape(1, DIM).astype(bf16)

    in_maps = []
    for c in range(NCORES):
        b, qt = c // QT, c % QT
        lo, hi = qt * T - HW, qt * T + T + HW
        xs = np.zeros((TH, DIM), dtype=np.float32)
        s0, s1 = max(lo, 0), min(hi, S)
        xs[s0 - lo:s1 - lo] = x[b, s0:s1]
        ind_r = np.zeros((1, TH), dtype=bf16)
        ind_r[0, s0 - lo:s1 - lo] = 1.0
        in_maps.append({
            "xT": np.ascontiguousarray(xs.T).astype(bf16),
            "Wq": wq_p, "Wk": wk_b, "Wv": wv_b, "Wo": wo_b,
            "bqc": bq_p, "bk_row": bk_r, "bv_row": bv_r, "bo_row": bo_r,
            "ind": ind_r,
        })
    return in_maps


def kernel(x, Wq, bq, Wk, bk, Wv, bv, Wo, bo):
    from concourse.bass_utils import run_bass_kernel_spmd

    nc = _build_nc()
    in_maps = _host_prep(x, Wq, bq, Wk, bk, Wv, bv, Wo, bo)
    res = run_bass_kernel_spmd(nc, in_maps, core_ids=list(range(NCORES)))
    out = np.empty((B, S, DIM), dtype=np.float32)
    for c in range(NCORES):
        b, qt = c // QT, c % QT
        out[b, qt * T:(qt + 1) * T] = res.results[c]["out"]
    return out
